# revision 4
# baseline (speedup 1.0000x reference)
"""Diagonalizable linear plant (modal state-space scan) on 8 Trainium2 cores.

y[b,t] = Cz @ z[b,t-1] + D @ u[b,t],  z[b,t] = lam * z[b,t-1] + Bz @ u[b,t]
with z[b,-1] = z0[b] = x0[b] @ Q, Bz = Q^T Bmat, Cz = C Q.

Sharding: data-parallel over batch (16 batches -> 2 per core). Host prep:
transpose u to (32, T) per batch so the contraction dim (n_u=32) lands on
SBUF partitions, pack small weights; device computes yT (32, T) per batch;
host transposes back.

Per-core device pipeline, chunked over T in L=1024 columns:
  PE  : V[h] (128, L) = BzT_h^T @ U          (float32r, 2 halves of 256 ch)
  DVE : Z[h] = tensor_tensor_scan(lam_h, V[h])   (the diagonal recurrence)
  ACT : stitch carry column, copy Y PSUM->SBUF
  PE  : Y (32, L) = CzT_h^T @ Zprev[h] (h=0,1) + D^T^T @ U
  DMA : U in, yT out
"""

import numpy as np

B, T, NX, NU, NY = 16, 8192, 256, 32, 32
NCORES = 8
BPC = B // NCORES  # batches per core
L = 1024           # time-chunk columns
NCHUNK = T // L

_PROG = None  # built Bass program, cached across kernel() calls


def _patch_tile_drain():
    """walrus codegen in this container rejects >1 sync wait on one SP
    TPB_CTRL instruction (terminal TileContext drain / NoOp). Split the
    drain's waits across preceding SP nops carrying one wait each."""
    import concourse.tile as tile
    import concourse.mybir as mybir
    from concourse.vector_clock import ScopedClock

    if getattr(tile.TileContext, "_drain_patched", False):
        return

    def _drain_and_barrier(self, tick_clock, wait_clock):
        nc = self.nc
        scratch = nc.sync.nop()
        wait_clock.add_sem_waits(
            scratch.ins, ScopedClock({None: tick_clock.global_clock})
        )
        si = scratch.ins.sync_info
        waits = list(si.on_wait) if si is not None else []
        scratch.ins.sync_info = mybir.SyncInfo(on_wait=waits[:1], on_update=[])
        for w in waits[1:]:
            n2 = nc.sync.nop()
            n2.ins.sync_info = mybir.SyncInfo(on_wait=[w], on_update=[])
        nc.sync.drain()
        nc.all_engine_barrier()
        assert self.sems is not None
        popped = nc._tile_sem_poison_stack.pop()
        assert popped is self._sem_poison
        nc.clear_and_free_semaphores(list(self.sems.allocated().values()))
        nc.all_engine_barrier()

    tile.TileContext._drain_and_barrier = _drain_and_barrier
    tile.TileContext._drain_patched = True


def _split_multi_waits(nc, mybir):
    """This container's walrus codegen accepts at most ONE sync wait per
    instruction. Hoist extra waits into standalone EventSemaphore nops on
    the same engine, placed immediately before the instruction."""
    ctr = [0]

    def fresh(engine, wait):
        ctr[0] += 1
        ev = mybir.InstEventSemaphore(name=f"I-wsplit-{ctr[0]}", ins=[], outs=[])
        ev.engine = engine
        ev.sync_info = mybir.SyncInfo(on_wait=[wait], on_update=[])
        nc.register_instruction(ev)
        return ev

    for fn in nc.m.functions:
        for bb in fn.blocks:
            out = []
            changed = False
            for inst in bb.instructions:
                si = inst.sync_info
                waits = list(si.on_wait) if si is not None else []
                if len(waits) > 1:
                    changed = True
                    for w in waits[:-1]:
                        out.append(fresh(inst.engine, w))
                    inst.sync_info = mybir.SyncInfo(
                        on_wait=[waits[-1]], on_update=list(si.on_update)
                    )
                out.append(inst)
            if changed:
                bb.instructions = out


def build_program():
    import concourse.bass as bass
    import concourse.tile as tile
    import concourse.mybir as mybir
    from contextlib import ExitStack

    _patch_tile_drain()
    f32 = mybir.dt.float32
    mmdt = mybir.dt.bfloat16

    nc = bass.Bass()
    uT = nc.declare_dram_parameter("uT", [BPC, NU, T], mmdt, isOutput=False)
    wBzT = nc.declare_dram_parameter("wBzT", [NU, NX], mmdt, isOutput=False)
    wCzD = nc.declare_dram_parameter("wCzD", [128, 96], mmdt, isOutput=False)
    z0c = nc.declare_dram_parameter("z0c", [128, 2 * BPC], f32, isOutput=False)
    lamb = nc.declare_dram_parameter("lamb", [128, 2], f32, isOutput=False)
    yT = nc.declare_dram_parameter("yT", [BPC, NY, T], f32, isOutput=True)

    with ExitStack() as ctx:
        tc = ctx.enter_context(tile.TileContext(nc))
        const = ctx.enter_context(tc.tile_pool(name="const", bufs=1))
        upool = ctx.enter_context(tc.tile_pool(name="u", bufs=3))
        vps = ctx.enter_context(tc.tile_pool(name="vps", bufs=2, space="PSUM"))
        yps = ctx.enter_context(tc.tile_pool(name="yps", bufs=2, space="PSUM"))
        zpool = ctx.enter_context(tc.tile_pool(name="z", bufs=6))
        yout = ctx.enter_context(tc.tile_pool(name="yo", bufs=3))

        bzT = const.tile([NU, NX], mmdt)
        nc.sync.dma_start(bzT[:], wBzT[:])
        czD = const.tile([128, 96], mmdt)
        nc.sync.dma_start(czD[:], wCzD[:])
        z0t = const.tile([128, 2 * BPC], f32)
        nc.sync.dma_start(z0t[:], z0c[:])
        lam2 = const.tile([128, 2], f32)
        nc.sync.dma_start(lam2[:], lamb[:])
        # lam broadcast tiles (128, L), one per channel half
        ones = const.tile([128, L], f32)
        nc.vector.memset(ones[:], 1.0)
        lam_bc = []
        for h in range(2):
            t = const.tile([128, L], f32, tag=f"lam_bc{h}")
            nc.vector.tensor_scalar_mul(t[:], ones[:], lam2[:, h : h + 1])
            lam_bc.append(t)

        mult = mybir.AluOpType.mult
        add = mybir.AluOpType.add

        for b in range(BPC):
            prev_z = [None, None]  # previous chunk's Z_ext tiles per half
            for c in range(NCHUNK):
                U = upool.tile([NU, L], mmdt)
                nc.sync.dma_start(U[:], uT[b, :, c * L : (c + 1) * L])

                zext = [None, None]
                for h in range(2):
                    V = vps.tile([128, L], f32)
                    for j in range(0, L, 512):
                        nc.tensor.matmul(
                            V[:, j : j + 512],
                            lhsT=bzT[:, h * 128 : (h + 1) * 128],
                            rhs=U[:, j : j + 512],
                            start=True,
                            stop=True,
                        )
                    Z = zpool.tile([128, L + 1], mmdt)
                    if c == 0:
                        carry = z0t[:, 2 * b + h : 2 * b + h + 1]
                    else:
                        carry = prev_z[h][:, L : L + 1]
                    nc.vector.tensor_tensor_scan(
                        Z[:, 1 : L + 1], lam_bc[h][:], V[:], carry, mult, add
                    )
                    nc.scalar.copy(Z[:, 0:1], carry)
                    zext[h] = Z

                Y = yps.tile([NY, L], f32)
                for j in range(0, L, 512):
                    sl = slice(j, j + 512)
                    nc.tensor.matmul(
                        Y[:, sl],
                        lhsT=czD[:, 0:32],
                        rhs=zext[0][:, sl],
                        start=True,
                        stop=False,
                    )
                    nc.tensor.matmul(
                        Y[:, sl],
                        lhsT=czD[:, 32:64],
                        rhs=zext[1][:, sl],
                        start=False,
                        stop=False,
                    )
                    nc.tensor.matmul(
                        Y[:, sl],
                        lhsT=czD[0:32, 64:96],
                        rhs=U[:, sl],
                        start=False,
                        stop=True,
                    )
                Ysb = yout.tile([NY, L], f32)
                nc.scalar.copy(Ysb[:], Y[:])
                nc.sync.dma_start(yT[b, :, c * L : (c + 1) * L], Ysb[:])
                prev_z = zext

    _split_multi_waits(nc, mybir)
    return nc


def _host_prep(x0, u, Q, lam, Bmat, C, D):
    import ml_dtypes
    f = np.float32
    bf = ml_dtypes.bfloat16
    Bz = (Q.T.astype(f) @ Bmat.astype(f)).astype(f)          # (NX, NU)
    BzT = np.ascontiguousarray(Bz.T).astype(bf)                          # (NU, NX)
    Cz = (C.astype(f) @ Q.astype(f)).astype(f)                # (NY, NX)
    CzT = Cz.T                                                # (NX, NY)
    wCzD = np.zeros((128, 96), dtype=bf)
    wCzD[:, 0:32] = CzT[0:128]
    wCzD[:, 32:64] = CzT[128:256]
    wCzD[0:32, 64:96] = D.T.astype(f)
    z0 = (x0.astype(f) @ Q.astype(f)).astype(f)               # (B, NX)
    uT = np.ascontiguousarray(u.transpose(0, 2, 1)).astype(bf)  # (B, NU, T)
    lamb = np.ascontiguousarray(
        np.stack([lam[:128], lam[128:]], axis=1)
    ).astype(f)                                               # (128, 2)
    return BzT, wCzD, z0, uT, lamb


def make_in_maps(x0, u, Q, lam, Bmat, C, D):
    BzT, wCzD, z0, uT, lamb = _host_prep(x0, u, Q, lam, Bmat, C, D)
    in_maps = []
    for cidx in range(NCORES):
        sl = slice(cidx * BPC, (cidx + 1) * BPC)
        z0_c = z0[sl]                                         # (BPC, NX)
        # (128, 2*BPC): col 2*b+h holds z0_c[b, 128h:128h+128]
        z0c = np.ascontiguousarray(
            z0_c.reshape(BPC, 2, 128).transpose(2, 0, 1).reshape(128, 2 * BPC)
        )
        in_maps.append(
            {
                "uT": np.ascontiguousarray(uT[sl]),
                "wBzT": BzT,
                "wCzD": wCzD,
                "z0c": z0c,
                "lamb": lamb,
            }
        )
    return in_maps


def kernel(x0, u, Q, lam, Bmat, C, D):
    global _PROG
    from concourse.bass_utils import run_bass_kernel_spmd

    if _PROG is None:
        _PROG = build_program()
    in_maps = make_in_maps(x0, u, Q, lam, Bmat, C, D)
    res = run_bass_kernel_spmd(_PROG, in_maps, list(range(NCORES)))
    y = np.empty((B, T, NY), dtype=np.float32)
    for cidx in range(NCORES):
        yT_c = res.results[cidx]["yT"]                        # (BPC, NY, T)
        y[cidx * BPC : (cidx + 1) * BPC] = yT_c.transpose(0, 2, 1)
    return y


# revision 6
# speedup vs baseline: 2.8052x; 2.8052x over previous
"""Diagonalizable linear plant (modal state-space scan) on 8 Trainium2 cores.

y[b,t] = Cz @ z[b,t-1] + D @ u[b,t],  z[b,t] = lam * z[b,t-1] + Bz @ u[b,t]
with z[b,-1] = z0[b] = x0[b] @ Q, Bz = Q^T Bmat, Cz = C Q.

Sharding: data-parallel over batch (16 batches -> 2 per core).

Block-4 formulation (the DVE scan instruction runs at ~2 cycles/element,
so the time axis is decimated 4x before it ever reaches the scan):
  host packs u as uT4[(i*32+u), k] = u[4k+i, u]   (K = 4 steps x 32 inputs = 128)
  PE   V4 = W2^T @ uT4        W2[(i,u),n] = lam_n^(3-i) Bz[n,u]   (block aggregate)
  DVE  zB = scan(lam^4, V4)   block-boundary states z_{4k+3}
  PE   Y4 = WC^T @ zBprev + WU^T @ uT4
       WC[n,(j,y)] = lam_n^j Cz[y,n]
       WU[(i,u),(j,y)] = (Cz lam^(j-1-i) Bz)[y,u] for i<j, D[y,u] for i=j, else 0
  host unpacks yT4[(32j+y), k] -> y[4k+j, y]

All matmuls are K=128, M=128, N=512 bf16 with fp32 PSUM accumulation.
"""

import numpy as np

B, T, NX, NU, NY = 16, 8192, 256, 32, 32
NCORES = 8
BPC = B // NCORES   # batches per core
MB = 4              # time-block folded into matmul K
K4 = T // MB        # block columns per batch (2048)
L = 512             # block-columns per chunk
NCHUNK = K4 // L    # chunks per batch (4)

_PROG = None  # built Bass program, cached across kernel() calls


def _patch_tile_drain():
    """walrus codegen in this container rejects >1 sync wait on one SP
    TPB_CTRL instruction (terminal TileContext drain / NoOp). Split the
    drain's waits across preceding SP nops carrying one wait each."""
    import concourse.tile as tile
    import concourse.mybir as mybir
    from concourse.vector_clock import ScopedClock

    if getattr(tile.TileContext, "_drain_patched", False):
        return

    def _drain_and_barrier(self, tick_clock, wait_clock):
        nc = self.nc
        scratch = nc.sync.nop()
        wait_clock.add_sem_waits(
            scratch.ins, ScopedClock({None: tick_clock.global_clock})
        )
        si = scratch.ins.sync_info
        waits = list(si.on_wait) if si is not None else []
        scratch.ins.sync_info = mybir.SyncInfo(on_wait=waits[:1], on_update=[])
        for w in waits[1:]:
            n2 = nc.sync.nop()
            n2.ins.sync_info = mybir.SyncInfo(on_wait=[w], on_update=[])
        nc.sync.drain()
        nc.all_engine_barrier()
        assert self.sems is not None
        popped = nc._tile_sem_poison_stack.pop()
        assert popped is self._sem_poison
        nc.clear_and_free_semaphores(list(self.sems.allocated().values()))
        nc.all_engine_barrier()

    tile.TileContext._drain_and_barrier = _drain_and_barrier
    tile.TileContext._drain_patched = True


def _split_multi_waits(nc, mybir):
    """This container's walrus codegen accepts at most ONE sync wait per
    instruction. Hoist extra waits into standalone EventSemaphore nops on
    the same engine, placed immediately before the instruction."""
    ctr = [0]

    def fresh(engine, wait):
        ctr[0] += 1
        ev = mybir.InstEventSemaphore(name=f"I-wsplit-{ctr[0]}", ins=[], outs=[])
        ev.engine = engine
        ev.sync_info = mybir.SyncInfo(on_wait=[wait], on_update=[])
        nc.register_instruction(ev)
        return ev

    for fn in nc.m.functions:
        for bb in fn.blocks:
            out = []
            changed = False
            for inst in bb.instructions:
                si = inst.sync_info
                waits = list(si.on_wait) if si is not None else []
                if len(waits) > 1:
                    changed = True
                    for w in waits[:-1]:
                        out.append(fresh(inst.engine, w))
                    inst.sync_info = mybir.SyncInfo(
                        on_wait=[waits[-1]], on_update=list(si.on_update)
                    )
                out.append(inst)
            if changed:
                bb.instructions = out


def build_program():
    import concourse.bass as bass
    import concourse.tile as tile
    import concourse.mybir as mybir
    from contextlib import ExitStack

    _patch_tile_drain()
    f32 = mybir.dt.float32
    bf = mybir.dt.bfloat16

    nc = bass.Bass()
    uT4 = nc.declare_dram_parameter("uT4", [BPC, 128, K4], bf, isOutput=False)
    wAll = nc.declare_dram_parameter("wAll", [128, 5 * 128], bf, isOutput=False)
    z0c = nc.declare_dram_parameter("z0c", [128, 2 * BPC], f32, isOutput=False)
    lam4b = nc.declare_dram_parameter("lam4b", [128, 2], f32, isOutput=False)
    yT4 = nc.declare_dram_parameter("yT4", [BPC, 128, K4], f32, isOutput=True)

    with ExitStack() as ctx:
        tc = ctx.enter_context(tile.TileContext(nc))
        const = ctx.enter_context(tc.tile_pool(name="const", bufs=1))
        upool = ctx.enter_context(tc.tile_pool(name="u", bufs=3))
        vps = ctx.enter_context(tc.tile_pool(name="vps", bufs=3, space="PSUM"))
        yps = ctx.enter_context(tc.tile_pool(name="yps", bufs=2, space="PSUM"))
        zpool = ctx.enter_context(tc.tile_pool(name="z", bufs=6))
        yout = ctx.enter_context(tc.tile_pool(name="yo", bufs=3))

        # weights: [W2_h0 | W2_h1 | WC_h0 | WC_h1 | WU] each (128,128)
        W = const.tile([128, 5 * 128], bf)
        nc.sync.dma_start(W[:], wAll[:])
        z0t = const.tile([128, 2 * BPC], f32)
        nc.sync.dma_start(z0t[:], z0c[:])
        lam2 = const.tile([128, 2], f32)
        nc.sync.dma_start(lam2[:], lam4b[:])
        ones = const.tile([128, L], f32)
        nc.vector.memset(ones[:], 1.0)
        lam_bc = []
        for h in range(2):
            t = const.tile([128, L], f32, tag=f"lam_bc{h}")
            nc.vector.tensor_scalar_mul(t[:], ones[:], lam2[:, h : h + 1])
            lam_bc.append(t)

        W2 = [W[:, 0:128], W[:, 128:256]]
        WC = [W[:, 256:384], W[:, 384:512]]
        WU = W[:, 512:640]

        mult = mybir.AluOpType.mult
        add = mybir.AluOpType.add

        for b in range(BPC):
            prev_z = [None, None]
            for c in range(NCHUNK):
                U = upool.tile([128, L], bf)
                nc.sync.dma_start(U[:], uT4[b, :, c * L : (c + 1) * L])

                zext = [None, None]
                for h in range(2):
                    V = vps.tile([128, L], f32)
                    nc.tensor.matmul(V[:], lhsT=W2[h], rhs=U[:], start=True, stop=True)
                    Z = zpool.tile([128, L + 1], bf)
                    if c == 0:
                        carry = z0t[:, 2 * b + h : 2 * b + h + 1]
                    else:
                        carry = prev_z[h][:, L : L + 1]
                    nc.vector.tensor_tensor_scan(
                        Z[:, 1 : L + 1], lam_bc[h][:], V[:], carry, mult, add
                    )
                    nc.scalar.copy(Z[:, 0:1], carry)
                    zext[h] = Z

                Y = yps.tile([128, L], f32)
                nc.tensor.matmul(Y[:], lhsT=WC[0], rhs=zext[0][:, 0:L],
                                 start=True, stop=False)
                nc.tensor.matmul(Y[:], lhsT=WC[1], rhs=zext[1][:, 0:L],
                                 start=False, stop=False)
                nc.tensor.matmul(Y[:], lhsT=WU, rhs=U[:], start=False, stop=True)

                Ysb = yout.tile([128, L], f32)
                nc.scalar.copy(Ysb[:], Y[:])
                nc.sync.dma_start(yT4[b, :, c * L : (c + 1) * L], Ysb[:])
                prev_z = zext

    _split_multi_waits(nc, mybir)
    return nc


def _host_prep(x0, u, Q, lam, Bmat, C, D):
    import ml_dtypes

    f = np.float32
    bf = ml_dtypes.bfloat16
    lam = lam.astype(f)
    Bz = (Q.T.astype(f) @ Bmat.astype(f)).astype(f)      # (NX, NU)
    Cz = (C.astype(f) @ Q.astype(f)).astype(f)           # (NY, NX)
    z0 = (x0.astype(f) @ Q.astype(f)).astype(f)          # (B, NX)

    lam_p = np.stack([np.ones_like(lam), lam, lam * lam, lam**3])  # (4, NX)

    # W2_h[(i*32+u), n] = lam_n^(3-i) * Bz[n, u]
    W2 = np.einsum("in,nu->iun", lam_p[::-1], Bz).reshape(MB * NU, NX)
    # WC_h[n, (32j+y)] = lam_n^j * Cz[y, n]
    WC = np.einsum("jn,yn->njy", lam_p, Cz).reshape(NX, MB * NY)
    # WU[(i*32+u), (32j+y)] = (Cz diag(lam^(j-1-i)) Bz)[y,u] i<j; D i=j; 0 i>j
    WU = np.zeros((MB * NU, MB * NY), dtype=f)
    for j in range(MB):
        for i in range(MB):
            if i < j:
                Mji = (Cz * lam_p[j - 1 - i][None, :]) @ Bz   # (NY, NU)
                WU[i * NU : (i + 1) * NU, j * NY : (j + 1) * NY] = Mji.T
            elif i == j:
                WU[i * NU : (i + 1) * NU, j * NY : (j + 1) * NY] = D.T.astype(f)

    wAll = np.zeros((128, 5 * 128), dtype=bf)
    wAll[:, 0:128] = W2[:, 0:128]
    wAll[:, 128:256] = W2[:, 128:256]
    wAll[0:128, 256:384] = WC[0:128]
    wAll[0:128, 384:512] = WC[128:256]
    wAll[:, 512:640] = WU

    # uT4[b][(i*32+u), k] = u[b, 4k+i, u]
    uT4 = np.ascontiguousarray(
        u.reshape(B, K4, MB, NU).transpose(0, 2, 3, 1).reshape(B, MB * NU, K4)
    ).astype(bf)

    lam4 = lam**MB
    lam4b = np.ascontiguousarray(np.stack([lam4[:128], lam4[128:]], axis=1)).astype(f)
    return wAll, z0, uT4, lam4b


def make_in_maps(x0, u, Q, lam, Bmat, C, D):
    wAll, z0, uT4, lam4b = _host_prep(x0, u, Q, lam, Bmat, C, D)
    in_maps = []
    for cidx in range(NCORES):
        sl = slice(cidx * BPC, (cidx + 1) * BPC)
        z0_c = z0[sl]
        z0c = np.ascontiguousarray(
            z0_c.reshape(BPC, 2, 128).transpose(2, 0, 1).reshape(128, 2 * BPC)
        )
        in_maps.append(
            {
                "uT4": np.ascontiguousarray(uT4[sl]),
                "wAll": wAll,
                "z0c": z0c,
                "lam4b": lam4b,
            }
        )
    return in_maps


def kernel(x0, u, Q, lam, Bmat, C, D):
    global _PROG
    from concourse.bass_utils import run_bass_kernel_spmd

    if _PROG is None:
        _PROG = build_program()
    in_maps = make_in_maps(x0, u, Q, lam, Bmat, C, D)
    res = run_bass_kernel_spmd(_PROG, in_maps, list(range(NCORES)))
    y = np.empty((B, T, NY), dtype=np.float32)
    for cidx in range(NCORES):
        yT4_c = res.results[cidx]["yT4"]                   # (BPC, 128, K4)
        # y[b, 4k+j, yy] = yT4[b, 32j+yy, k]
        y[cidx * BPC : (cidx + 1) * BPC] = (
            yT4_c.reshape(BPC, MB, NY, K4).transpose(0, 3, 1, 2).reshape(BPC, T, NY)
        )
    return y
